# revision 20
# baseline (speedup 1.0000x reference)
"""Self-attention (SAGAN-style) Trainium2 kernel, v5.

Reference computation (per batch sample):
    theta = w_theta @ x            # [32, 4096]
    phi   = pool2x2(w_phi @ x)     # [32, 1024]
    g     = pool2x2(w_g @ x)       # [128, 1024]
    beta  = softmax(theta.T @ phi, axis=-1)   # [4096, 1024]
    attn  = g @ beta.T             # [128, 4096]
    out   = gamma * (w_o @ attn) + x

Sharding: data-parallel over batch; B=16 over 8 cores -> 2 samples/core.

v5 schedule (per core, per sample):
  - all weights packed into ONE casting DMA; first x chunk DMA issued
    before everything else on the SWDGE queue (lead-in shave).
  - x bf16 via casting DMAs; bf16 residual.
  - qt-0 score rounds interleaved into the projection loop; th evac on
    ScalarE; fused 2x2 pool evac (tensor_reduce XY) on DVE.
  - score rounds emitted as [spa-pair, exp(spa), pop, spb-pair, exp(spb),
    pop]: the PE's in-order queue never waits on the staggered exp/PSUM
    recycle, because popped work units sit between the half-rounds.
  - work units fine-grained (attn 8MM / den 8MM+recip+norm / out 2MM+
    residual+store) with the out unit lagged so the PE never waits on the
    DVE normalize chain; pops are gated on the unit's quarter being fully
    exponentiated so early pops cannot stall the in-order PE queue.
  - output stores on sync HWDGE; gts double-buffered across samples.
"""

import numpy as np

import concourse.bacc as bacc
import concourse.mybir as mybir
from concourse import tile
from concourse.bass_utils import run_bass_kernel_spmd

F32 = mybir.dt.float32
BF16 = mybir.dt.bfloat16

B, C, H, W = 16, 256, 64, 64
N = H * W            # 4096
M = N // 4           # 1024
C8 = C // 8          # 32
C2 = C // 2          # 128
NCORES = 8
BPC = B // NCORES    # 2 samples per core
NCH = 512            # n-chunk width for matmul streaming
NNCH = N // NCH      # 8
MC = M // 128        # 8 m-chunks


def build_kernel():
    nc = bacc.Bacc("TRN2", target_bir_lowering=False, debug=False)

    x_d = nc.declare_dram_parameter("x", [BPC, C, N], F32, isOutput=False)
    # packed weights: [wq0|wq1|wg0|wg1|wo|ident] along free dim
    wc_d = nc.declare_dram_parameter("wc", [128, 896], F32, isOutput=False)
    out_d = nc.declare_dram_parameter("out", [BPC, C, N], F32, isOutput=True)

    with tile.TileContext(nc) as tc:
        with (
            tc.tile_pool(name="const", bufs=1) as constp,
            tc.tile_pool(name="xsb", bufs=2) as xp,
            tc.tile_pool(name="proj", bufs=1) as projp,
            tc.tile_pool(name="exp", bufs=1) as expp,
            tc.tile_pool(name="gt", bufs=2) as gtp,
            tc.tile_pool(name="small", bufs=3) as smallp,
            tc.tile_pool(name="outs", bufs=3) as outp,
            tc.tile_pool(name="ps_big", bufs=2, space="PSUM") as psS,
            tc.tile_pool(name="ps_a", bufs=1, space="PSUM") as psA,
            tc.tile_pool(name="ps_d", bufs=1, space="PSUM") as psD,
            tc.tile_pool(name="ps_o", bufs=1, space="PSUM") as psO,
        ):
            # first x chunk before anything else on the SWDGE queue
            xbfs = []
            for b in range(BPC):
                xbfs.append(xp.tile([128, 2, N], BF16, tag="xbf",
                                    name=f"xbf_{b}"))
            wc = constp.tile([128, 896], BF16, tag="wc")
            nc.gpsimd.dma_start(wc[:], wc_d[:])
            xdv0 = x_d[0].rearrange("(cc p) n -> p cc n", p=128)
            nc.gpsimd.dma_start(xbfs[0][:, :, 0:512], xdv0[:, :, 0:512])
            wq = [wc[:, 0:128], wc[:, 128:256]]
            wg = [wc[:, 256:384], wc[:, 384:512]]
            wo = wc[:, 512:768]
            id_b = wc[:, 768:896]
            ones = constp.tile([128, 128], BF16, tag="ones")
            nc.gpsimd.memset(ones[:], 1.0)

            # pending work units: (min_quarter, fn); pop only when the
            # needed quarter is fully exponentiated
            pending = []

            def pop_unit(qt_done=99, limit=1):
                n = 0
                while pending and n < limit and pending[0][0] <= qt_done:
                    pending.pop(0)[1]()
                    n += 1

            for b in range(BPC):
                xbf = xbfs[b]
                xdv = x_d[b].rearrange("(cc p) n -> p cc n", p=128)
                slices = ((slice(512, 1536), slice(1536, 2816),
                           slice(2816, 4096)) if b == 0 else
                          (slice(0, 1024), slice(1024, 2048),
                           slice(2048, 3072), slice(3072, 4096)))
                for sl in slices:
                    nc.gpsimd.dma_start(xbf[:, :, sl], xdv[:, :, sl])

                th4 = projp.tile([64, N], BF16, tag="th4", name=f"th4_{b}")
                ph4 = projp.tile([64, M], BF16, tag="ph4", name=f"ph4_{b}")
                gp = projp.tile([C2, M], BF16, tag="g_p", name=f"gp_{b}")
                gts = []
                ets = []
                for mc in range(MC):
                    et = expp.tile([128, N], BF16, tag=f"expT{mc}",
                                   name=f"expT{mc}_{b}")
                    ets.append(et)

                at_map = {}

                def unit_attn(i, b=b, ets=ets, gts=gts, at_map=at_map):
                    nsl = slice(i * NCH, (i + 1) * NCH)
                    aps = psA.tile([128, NCH], F32, tag="a", name=f"aps{b}_{i}")
                    at_map[(i, 'a')] = aps
                    for mc in range(MC):
                        nc.tensor.matmul(aps[:], gts[mc][:], ets[mc][:, nsl],
                                         start=(mc == 0), stop=(mc == MC - 1),
                                         skip_group_check=True)

                def unit_den(i, b=b, ets=ets, at_map=at_map):
                    nsl = slice(i * NCH, (i + 1) * NCH)
                    dps = psD.tile([128, NCH], F32, tag="d", name=f"dps{b}_{i}")
                    for mc in range(MC):
                        nc.tensor.matmul(dps[:], ones[:], ets[mc][:, nsl],
                                         start=(mc == 0), stop=(mc == MC - 1),
                                         skip_group_check=True)
                    aps = at_map.pop((i, 'a'))
                    rec = smallp.tile([128, NCH], F32, tag="rec",
                                      name=f"rec{b}_{i}")
                    nc.vector.reciprocal_approx_fast(rec[:], dps[:])
                    at = smallp.tile([128, NCH], BF16, tag="attn",
                                     name=f"at{b}_{i}")
                    nc.vector.scalar_tensor_tensor(
                        at[:], aps[:], 1.0, rec[:],
                        mybir.AluOpType.bypass, mybir.AluOpType.mult)
                    at_map[(i, 't')] = at

                def unit_out(i, b=b, xbf=xbf, at_map=at_map):
                    nsl = slice(i * NCH, (i + 1) * NCH)
                    at = at_map.pop((i, 't'))
                    pso = psO.tile([128, 2, NCH], F32, tag="o", name=f"pso{b}_{i}")
                    nc.tensor.matmul(pso[:, 0], wo[:, 0:128], at[:],
                                     start=True, stop=True)
                    nc.tensor.matmul(pso[:, 1], wo[:, 128:256], at[:],
                                     start=True, stop=True)
                    osb = outp.tile([128, 2, NCH], F32, tag="osb",
                                    name=f"osb{b}_{i}")
                    nc.vector.scalar_tensor_tensor(
                        osb[:], pso[:], 1.0, xbf[:, :, nsl],
                        mybir.AluOpType.bypass, mybir.AluOpType.add)
                    # out[b, (cc p), nsl] <- osb[p, cc, :]
                    ov = out_d[b, :, nsl].rearrange("(cc p) n -> p cc n", p=128)
                    nc.sync.dma_start(ov, osb[:])

                def emit_round(qt, r, qt_done, b=b, ets=ets, th4=th4, ph4=ph4):
                    mc_a, mc_b = 2 * r, 2 * r + 1
                    ca = slice(mc_a * 128, (mc_a + 1) * 128)
                    cb = slice(mc_b * 128, (mc_b + 1) * 128)
                    qsl = slice(qt * 1024, (qt + 1) * 1024)
                    h0 = slice(qt * 1024, qt * 1024 + 512)
                    h1 = slice(qt * 1024 + 512, (qt + 1) * 1024)
                    spa = psS.tile([128, 1024], F32, tag="big",
                                   name=f"spa{b}_{qt}_{r}")
                    nc.tensor.matmul(spa[:, 0:512], ph4[0:32, ca],
                                     th4[0:32, h0], start=True, stop=True,
                                     tile_position=(0, 0))
                    nc.tensor.matmul(spa[:, 512:1024], ph4[32:64, ca],
                                     th4[32:64, h1], start=True, stop=True,
                                     tile_position=(32, 0))
                    nc.scalar.activation(ets[mc_a][:, qsl], spa[:],
                                         mybir.ActivationFunctionType.Exp)
                    pop_unit(qt_done)
                    spb = psS.tile([128, 1024], F32, tag="big",
                                   name=f"spb{b}_{qt}_{r}")
                    nc.tensor.matmul(spb[:, 0:512], ph4[0:32, cb],
                                     th4[0:32, h0], start=True, stop=True,
                                     tile_position=(0, 0))
                    nc.tensor.matmul(spb[:, 512:1024], ph4[32:64, cb],
                                     th4[32:64, h1], start=True, stop=True,
                                     tile_position=(32, 0))
                    nc.scalar.activation(ets[mc_b][:, qsl], spb[:],
                                         mybir.ActivationFunctionType.Exp)
                    pop_unit(qt_done)

                # ---- projection loop with interleaved qt0 rounds ----
                for i in range(NNCH):
                    sl = slice(i * NCH, (i + 1) * NCH)
                    msl = slice(i * 128, (i + 1) * 128)
                    ps1 = psS.tile([128, NCH], F32, tag="big", name=f"ps1_{b}_{i}")
                    for cc in range(2):
                        nc.tensor.matmul(ps1[:], wq[cc], xbf[:, cc, sl],
                                         start=(cc == 0), stop=(cc == 1))
                    # theta rows 0:64 -> SBUF bf16 (ScalarE); phi 2x2-pooled
                    nc.scalar.copy(th4[0:64, sl], ps1[0:64])
                    pv = ps1[64:128].rearrange(
                        "p (h2 hb w2 wb) -> p h2 w2 hb wb", h2=4, hb=2, wb=2)
                    nc.vector.tensor_reduce(ph4[0:64, msl], pv[:],
                                            mybir.AxisListType.XY,
                                            mybir.AluOpType.max)
                    ps2 = psS.tile([128, NCH], F32, tag="big", name=f"ps2_{b}_{i}")
                    for cc in range(2):
                        nc.tensor.matmul(ps2[:], wg[cc], xbf[:, cc, sl],
                                         start=(cc == 0), stop=(cc == 1))
                    pv2 = ps2[:].rearrange(
                        "p (h2 hb w2 wb) -> p h2 w2 hb wb", h2=4, hb=2, wb=2)
                    nc.vector.tensor_reduce(gp[:, msl], pv2[:],
                                            mybir.AxisListType.XY,
                                            mybir.AluOpType.max)
                    pop_unit()

                    def emit_transpose(mc, b=b, gp=gp, gts=gts):
                        tp = psD.tile([128, 128], BF16, tag="d",
                                      name=f"tp{b}_{mc}")
                        nc.tensor.transpose(tp[:], gp[:, mc * 128:(mc + 1) * 128],
                                            id_b[:])
                        gt = gtp.tile([128, 128], BF16, tag=f"gt{mc}",
                                      name=f"gt{mc}_{b}")
                        nc.scalar.copy(gt[:], tp[:])
                        gts.append(gt)

                    # transpose/round for the PREVIOUS chunk(s): their pool
                    # outputs are guaranteed ready, so the PE never stalls
                    if i >= 1:
                        emit_transpose(i - 1)
                    if i >= 2 and i % 2 == 0:
                        emit_round(0, (i - 2) // 2, qt_done=99 if b else -1)
                if True:
                    emit_transpose(NNCH - 1)
                    emit_round(0, 3, qt_done=99 if b else -1)

                # queue this sample's work units (quarter-gated pops)
                for i in range(NNCH):
                    q = i // 2
                    pending.append((q, lambda f=unit_attn, i=i: f(i)))
                    pending.append((q, lambda f=unit_den, i=i: f(i)))
                    if i >= 2:
                        pending.append((i // 2 - 1,
                                        lambda f=unit_out, i=i - 2: f(i)))
                pending.append((3, lambda f=unit_out, i=NNCH - 2: f(i)))
                pending.append((3, lambda f=unit_out, i=NNCH - 1: f(i)))

                # ---- remaining quarters ----
                for qt in range(1, 4):
                    for r in range(4):
                        emit_round(qt, r, qt_done=qt - 1)

                if b == BPC - 1:
                    while pending:
                        pop_unit()

    nc.compile()
    return nc


_NC_CACHE = None


def _get_nc():
    global _NC_CACHE
    if _NC_CACHE is None:
        _NC_CACHE = build_kernel()
    return _NC_CACHE


def prep_inputs(x, w_theta, w_phi, w_g, w_o, gamma):
    """Host-side prep: shard x over 8 cores; pack/scale weights."""
    x = np.asarray(x, dtype=np.float32).reshape(B, C, N)
    w_theta = np.asarray(w_theta, dtype=np.float32)
    w_phi = np.asarray(w_phi, dtype=np.float32)
    w_g = np.asarray(w_g, dtype=np.float32)
    w_o = np.asarray(w_o, dtype=np.float32)
    gamma = np.float32(gamma)

    # combined projection weight: [th th ph ph] along output dim
    wqT = np.concatenate([w_theta.T, w_theta.T, w_phi.T, w_phi.T], axis=1)  # [256,128]
    wq = wqT.reshape(2, 128, 128)
    wgq = w_g.T.reshape(2, 128, C2)
    wo = (gamma * w_o).T                       # [128, 256]
    ident = np.eye(128, dtype=np.float32)
    wc = np.ascontiguousarray(np.concatenate(
        [wq[0], wq[1], wgq[0], wgq[1], wo, ident], axis=1))  # [128, 896]

    in_maps = []
    for core in range(NCORES):
        shard = np.ascontiguousarray(x[core * BPC:(core + 1) * BPC])
        in_maps.append({"x": shard, "wc": wc})
    return in_maps


def run(inputs, trace=False, **kw):
    nc = _get_nc()
    in_maps = prep_inputs(**inputs)
    res = run_bass_kernel_spmd(nc, in_maps, core_ids=list(range(NCORES)),
                               trace=trace, **kw)
    outs = [res.results[i]["out"] for i in range(NCORES)]
    full = np.concatenate(outs, axis=0).reshape(B, C, H, W).astype(np.float32)
    return full, res


def kernel(**inputs):
    full, _ = run(inputs, trace=False)
    return full


# revision 22
# speedup vs baseline: 1.0268x; 1.0268x over previous
"""Self-attention (SAGAN-style) Trainium2 kernel, v5.

Reference computation (per batch sample):
    theta = w_theta @ x            # [32, 4096]
    phi   = pool2x2(w_phi @ x)     # [32, 1024]
    g     = pool2x2(w_g @ x)       # [128, 1024]
    beta  = softmax(theta.T @ phi, axis=-1)   # [4096, 1024]
    attn  = g @ beta.T             # [128, 4096]
    out   = gamma * (w_o @ attn) + x

Sharding: data-parallel over batch; B=16 over 8 cores -> 2 samples/core.

v5 schedule (per core, per sample):
  - all weights packed into ONE casting DMA; first x chunk DMA issued
    before everything else on the SWDGE queue (lead-in shave).
  - x bf16 via casting DMAs; bf16 residual.
  - qt-0 score rounds interleaved into the projection loop; th evac on
    ScalarE; fused 2x2 pool evac (tensor_reduce XY) on DVE.
  - score rounds emitted as [spa-pair, exp(spa), pop, spb-pair, exp(spb),
    pop]: the PE's in-order queue never waits on the staggered exp/PSUM
    recycle, because popped work units sit between the half-rounds.
  - work units fine-grained (attn 8MM / den 8MM+recip+norm / out 2MM+
    residual+store) with the out unit lagged so the PE never waits on the
    DVE normalize chain; pops are gated on the unit's quarter being fully
    exponentiated so early pops cannot stall the in-order PE queue.
  - output stores on sync HWDGE; gts double-buffered across samples.
"""

import numpy as np

import concourse.bacc as bacc
import concourse.mybir as mybir
from concourse import tile
from concourse.bass_utils import run_bass_kernel_spmd

F32 = mybir.dt.float32
BF16 = mybir.dt.bfloat16

B, C, H, W = 16, 256, 64, 64
N = H * W            # 4096
M = N // 4           # 1024
C8 = C // 8          # 32
C2 = C // 2          # 128
NCORES = 8
BPC = B // NCORES    # 2 samples per core
NCH = 512            # n-chunk width for matmul streaming
NNCH = N // NCH      # 8
MC = M // 128        # 8 m-chunks


def build_kernel():
    nc = bacc.Bacc("TRN2", target_bir_lowering=False, debug=False)

    x_d = nc.declare_dram_parameter("x", [BPC, C, N], F32, isOutput=False)
    # packed weights: [wq0|wq1|wg0|wg1|wo|ident] along free dim
    wc_d = nc.declare_dram_parameter("wc", [128, 896], F32, isOutput=False)
    out_d = nc.declare_dram_parameter("out", [BPC, C, N], F32, isOutput=True)

    with tile.TileContext(nc) as tc:
        with (
            tc.tile_pool(name="const", bufs=1) as constp,
            tc.tile_pool(name="xsb", bufs=2) as xp,
            tc.tile_pool(name="proj", bufs=1) as projp,
            tc.tile_pool(name="exp", bufs=1) as expp,
            tc.tile_pool(name="gt", bufs=2) as gtp,
            tc.tile_pool(name="small", bufs=3) as smallp,
            tc.tile_pool(name="outs", bufs=3) as outp,
            tc.tile_pool(name="ps_big", bufs=2, space="PSUM") as psS,
            tc.tile_pool(name="ps_a", bufs=1, space="PSUM") as psA,
            tc.tile_pool(name="ps_d", bufs=1, space="PSUM") as psD,
            tc.tile_pool(name="ps_o", bufs=1, space="PSUM") as psO,
        ):
            # first x chunk before anything else on the SWDGE queue
            xbfs = []
            for b in range(BPC):
                xbfs.append(xp.tile([128, 2, N], BF16, tag="xbf",
                                    name=f"xbf_{b}"))
            wc = constp.tile([128, 896], BF16, tag="wc")
            nc.gpsimd.dma_start(wc[:], wc_d[:])
            xdv0 = x_d[0].rearrange("(cc p) n -> p cc n", p=128)
            nc.gpsimd.dma_start(xbfs[0][:, :, 0:512], xdv0[:, :, 0:512])
            wq = [wc[:, 0:128], wc[:, 128:256]]
            wg = [wc[:, 256:384], wc[:, 384:512]]
            wo = wc[:, 512:768]
            id_b = wc[:, 768:896]
            ones = constp.tile([128, 128], BF16, tag="ones")
            nc.gpsimd.memset(ones[:], 1.0)

            # pending work units: (min_quarter, fn); pop only when the
            # needed quarter is fully exponentiated
            pending = []

            def pop_unit(qt_done=99, limit=1):
                n = 0
                while pending and n < limit and pending[0][0] <= qt_done:
                    pending.pop(0)[1]()
                    n += 1

            for b in range(BPC):
                xbf = xbfs[b]
                xdv = x_d[b].rearrange("(cc p) n -> p cc n", p=128)
                slices = ((slice(512, 1536), slice(1536, 2816),
                           slice(2816, 4096)) if b == 0 else
                          (slice(0, 1024), slice(1024, 2048),
                           slice(2048, 3072), slice(3072, 4096)))
                for sl in slices:
                    nc.gpsimd.dma_start(xbf[:, :, sl], xdv[:, :, sl])

                th4 = projp.tile([128, N], BF16, tag="th4", name=f"th4_{b}")
                ph4 = projp.tile([128, M], BF16, tag="ph4", name=f"ph4_{b}")
                gp = projp.tile([C2, M], BF16, tag="g_p", name=f"gp_{b}")
                gts = []
                ets = []
                for mc in range(MC):
                    et = expp.tile([128, N], BF16, tag=f"expT{mc}",
                                   name=f"expT{mc}_{b}")
                    ets.append(et)

                at_map = {}

                def unit_attn(i, b=b, ets=ets, gts=gts, at_map=at_map):
                    nsl = slice(i * NCH, (i + 1) * NCH)
                    aps = psA.tile([128, NCH], F32, tag="a", name=f"aps{b}_{i}")
                    at_map[(i, 'a')] = aps
                    for mc in range(MC):
                        nc.tensor.matmul(aps[:], gts[mc][:], ets[mc][:, nsl],
                                         start=(mc == 0), stop=(mc == MC - 1),
                                         skip_group_check=True)

                def unit_den(i, b=b, ets=ets, at_map=at_map):
                    nsl = slice(i * NCH, (i + 1) * NCH)
                    dps = psD.tile([128, NCH], F32, tag="d", name=f"dps{b}_{i}")
                    for mc in range(MC):
                        nc.tensor.matmul(dps[:], ones[:], ets[mc][:, nsl],
                                         start=(mc == 0), stop=(mc == MC - 1),
                                         skip_group_check=True)
                    aps = at_map.pop((i, 'a'))
                    rec = smallp.tile([128, NCH], F32, tag="rec",
                                      name=f"rec{b}_{i}")
                    nc.vector.reciprocal_approx_fast(rec[:], dps[:])
                    at = smallp.tile([128, NCH], BF16, tag="attn",
                                     name=f"at{b}_{i}")
                    nc.vector.scalar_tensor_tensor(
                        at[:], aps[:], 1.0, rec[:],
                        mybir.AluOpType.bypass, mybir.AluOpType.mult)
                    at_map[(i, 't')] = at

                def unit_out(i, b=b, xbf=xbf, at_map=at_map):
                    nsl = slice(i * NCH, (i + 1) * NCH)
                    at = at_map.pop((i, 't'))
                    pso = psO.tile([128, 2, NCH], F32, tag="o", name=f"pso{b}_{i}")
                    nc.tensor.matmul(pso[:, 0], wo[:, 0:128], at[:],
                                     start=True, stop=True)
                    nc.tensor.matmul(pso[:, 1], wo[:, 128:256], at[:],
                                     start=True, stop=True)
                    osb = outp.tile([128, 2, NCH], F32, tag="osb",
                                    name=f"osb{b}_{i}")
                    nc.vector.scalar_tensor_tensor(
                        osb[:], pso[:], 1.0, xbf[:, :, nsl],
                        mybir.AluOpType.bypass, mybir.AluOpType.add)
                    # out[b, (cc p), nsl] <- osb[p, cc, :]
                    ov = out_d[b, :, nsl].rearrange("(cc p) n -> p cc n", p=128)
                    nc.sync.dma_start(ov, osb[:])

                def emit_round(qt, r, qt_done, b=b, ets=ets, th4=th4, ph4=ph4):
                    mc_a, mc_b = 2 * r, 2 * r + 1
                    ca = slice(mc_a * 128, (mc_a + 1) * 128)
                    cb = slice(mc_b * 128, (mc_b + 1) * 128)
                    qsl = slice(qt * 1024, (qt + 1) * 1024)
                    h0 = slice(qt * 1024, qt * 1024 + 512)
                    h1 = slice(qt * 1024 + 512, (qt + 1) * 1024)
                    spa = psS.tile([128, 1024], F32, tag="big",
                                   name=f"spa{b}_{qt}_{r}")
                    nc.tensor.matmul(spa[:, 0:512], ph4[0:32, ca],
                                     th4[0:32, h0], start=True, stop=True,
                                     tile_position=(0, 0))
                    nc.tensor.matmul(spa[:, 512:1024], ph4[64:96, ca],
                                     th4[64:96, h1], start=True, stop=True,
                                     tile_position=(64, 0))
                    nc.scalar.activation(ets[mc_a][:, qsl], spa[:],
                                         mybir.ActivationFunctionType.Exp)
                    pop_unit(qt_done)
                    spb = psS.tile([128, 1024], F32, tag="big",
                                   name=f"spb{b}_{qt}_{r}")
                    nc.tensor.matmul(spb[:, 0:512], ph4[32:64, cb],
                                     th4[32:64, h0], start=True, stop=True,
                                     tile_position=(32, 0))
                    nc.tensor.matmul(spb[:, 512:1024], ph4[96:128, cb],
                                     th4[96:128, h1], start=True, stop=True,
                                     tile_position=(96, 0))
                    nc.scalar.activation(ets[mc_b][:, qsl], spb[:],
                                         mybir.ActivationFunctionType.Exp)
                    pop_unit(qt_done)

                # ---- projection loop with interleaved qt0 rounds ----
                for i in range(NNCH):
                    sl = slice(i * NCH, (i + 1) * NCH)
                    msl = slice(i * 128, (i + 1) * 128)
                    ps1 = psS.tile([128, NCH], F32, tag="big", name=f"ps1_{b}_{i}")
                    for cc in range(2):
                        nc.tensor.matmul(ps1[:], wq[cc], xbf[:, cc, sl],
                                         start=(cc == 0), stop=(cc == 1))
                    # theta rows 0:64 -> SBUF bf16; phi 2x2-pooled.
                    # Sample 0's proj phase is ScalarE-paced (no drain chains
                    # on DVE yet), so its evac goes to DVE instead
                    if b == 0:
                        nc.vector.tensor_copy(th4[0:64, sl], ps1[0:64])
                    else:
                        nc.scalar.copy(th4[0:64, sl], ps1[0:64])
                    pv = ps1[64:128].rearrange(
                        "p (h2 hb w2 wb) -> p h2 w2 hb wb", h2=4, hb=2, wb=2)
                    nc.vector.tensor_reduce(ph4[0:64, msl], pv[:],
                                            mybir.AxisListType.XY,
                                            mybir.AluOpType.max)
                    ps2 = psS.tile([128, NCH], F32, tag="big", name=f"ps2_{b}_{i}")
                    for cc in range(2):
                        nc.tensor.matmul(ps2[:], wg[cc], xbf[:, cc, sl],
                                         start=(cc == 0), stop=(cc == 1))
                    pv2 = ps2[:].rearrange(
                        "p (h2 hb w2 wb) -> p h2 w2 hb wb", h2=4, hb=2, wb=2)
                    nc.vector.tensor_reduce(gp[:, msl], pv2[:],
                                            mybir.AxisListType.XY,
                                            mybir.AluOpType.max)
                    # 4-way duplicates for this chunk (ScalarE: idle in
                    # proj phases while DVE paces the pools)
                    nc.scalar.copy(th4[64:128, sl], th4[0:64, sl])
                    nc.scalar.copy(ph4[64:128, msl], ph4[0:64, msl])
                    pop_unit()

                    def emit_transpose(mc, b=b, gp=gp, gts=gts):
                        tp = psD.tile([128, 128], BF16, tag="d",
                                      name=f"tp{b}_{mc}")
                        nc.tensor.transpose(tp[:], gp[:, mc * 128:(mc + 1) * 128],
                                            id_b[:])
                        gt = gtp.tile([128, 128], BF16, tag=f"gt{mc}",
                                      name=f"gt{mc}_{b}")
                        nc.scalar.copy(gt[:], tp[:])
                        gts.append(gt)

                    # transpose/round for the PREVIOUS chunk(s): their pool
                    # outputs are guaranteed ready, so the PE never stalls
                    if i >= 1:
                        emit_transpose(i - 1)
                    if i >= 2 and i % 2 == 0:
                        emit_round(0, (i - 2) // 2, qt_done=99 if b else -1)
                if True:
                    emit_transpose(NNCH - 1)
                    emit_round(0, 3, qt_done=99 if b else -1)

                # queue this sample's work units (quarter-gated pops)
                for i in range(NNCH):
                    q = i // 2
                    pending.append((q, lambda f=unit_attn, i=i: f(i)))
                    pending.append((q, lambda f=unit_den, i=i: f(i)))
                    if i >= 2:
                        pending.append((i // 2 - 1,
                                        lambda f=unit_out, i=i - 2: f(i)))
                pending.append((3, lambda f=unit_out, i=NNCH - 2: f(i)))
                pending.append((3, lambda f=unit_out, i=NNCH - 1: f(i)))

                # ---- remaining quarters ----
                for qt in range(1, 4):
                    for r in range(4):
                        emit_round(qt, r, qt_done=qt - 1)

                if b == BPC - 1:
                    while pending:
                        pop_unit()

    nc.compile()
    return nc


_NC_CACHE = None


def _get_nc():
    global _NC_CACHE
    if _NC_CACHE is None:
        _NC_CACHE = build_kernel()
    return _NC_CACHE


def prep_inputs(x, w_theta, w_phi, w_g, w_o, gamma):
    """Host-side prep: shard x over 8 cores; pack/scale weights."""
    x = np.asarray(x, dtype=np.float32).reshape(B, C, N)
    w_theta = np.asarray(w_theta, dtype=np.float32)
    w_phi = np.asarray(w_phi, dtype=np.float32)
    w_g = np.asarray(w_g, dtype=np.float32)
    w_o = np.asarray(w_o, dtype=np.float32)
    gamma = np.float32(gamma)

    # combined projection weight: [th th ph ph] along output dim
    wqT = np.concatenate([w_theta.T, w_theta.T, w_phi.T, w_phi.T], axis=1)  # [256,128]
    wq = wqT.reshape(2, 128, 128)
    wgq = w_g.T.reshape(2, 128, C2)
    wo = (gamma * w_o).T                       # [128, 256]
    ident = np.eye(128, dtype=np.float32)
    wc = np.ascontiguousarray(np.concatenate(
        [wq[0], wq[1], wgq[0], wgq[1], wo, ident], axis=1))  # [128, 896]

    in_maps = []
    for core in range(NCORES):
        shard = np.ascontiguousarray(x[core * BPC:(core + 1) * BPC])
        in_maps.append({"x": shard, "wc": wc})
    return in_maps


def run(inputs, trace=False, **kw):
    nc = _get_nc()
    in_maps = prep_inputs(**inputs)
    res = run_bass_kernel_spmd(nc, in_maps, core_ids=list(range(NCORES)),
                               trace=trace, **kw)
    outs = [res.results[i]["out"] for i in range(NCORES)]
    full = np.concatenate(outs, axis=0).reshape(B, C, H, W).astype(np.float32)
    return full, res


def kernel(**inputs):
    full, _ = run(inputs, trace=False)
    return full
